# revision 18
# baseline (speedup 1.0000x reference)
"""Masked multi-head attention kernel for 8 Trainium2 NeuronCores.

Strategy (v2 — PE-packing rework of the v1 baseline):
  - 24 (batch, head) pairs sharded as: core c -> batch c//4, heads [3*(c%4) .. 3*(c%4)+2].
  - Key-padding mask handled by HOST-side gather: only unmasked key positions are
    shipped/computed. Padded key slots get zeroed K columns (scores=0 -> exp=1)
    and a 0 in the indicator slot of V, so they contribute nothing.
  - Softmax without max-subtraction (scores ~ N(0,1); masked keys excluded).
  - Row-sum of exp folded into the AV matmul via an indicator slot on V.
  - PE tile packing (the big v2 win): all D=64-contraction matmuls run as
    concurrent PAIRS in the two 64-row halves of the PE array
    (tile_position auto-derived from partition bases):
      * scores: head0 (partitions 0-63) paired with head1 (64-127) per key
        block; head2 paired with itself via duplicated kT2/qT2 (dup'd by
        SBUF->SBUF DMA, free on the DMA engines).
      * out-proj: the K=64 tail (head2 dims) packed as column-group pairs
        (even cg rows 0-63, odd cg rows 64-127) against duplicated OT2.
      * qT2/kT2 projections: M=64 outputs packed 2 query-chunks per pass
        via column tiling.
  - One exp per score pair: [128, 2*512] ACTIVATE (amortizes ACT overhead,
    covers both heads of the pair).
  - ACT table preloaded by a dummy exp at t=0; all DMAs issued on the
    Sync + GpSimd queues so the Scalar engine does nothing but exp.
  - V indicator slots built on-device (gpsimd memset) + a tiny DMA for the
    padded tail block.
  - PSUM: 4 banks score pairs (x2), 2 banks AV accumulators, 2 banks proj.
  - bf16 matmul inputs, fp32 PSUM accumulation, bf16 output partials
    (host sums the 4 partials per batch in fp32, adds proj_b).
"""

import math

import numpy as np
import ml_dtypes

BF16 = ml_dtypes.bfloat16
B, N, C = 2, 2048, 768
H = 12
D = 64
HPC = 3          # heads per core
P = 128
QB = 512         # query block
NQB = N // QB
SCALE = D ** -0.5
NCORES = 8


def _chunks(total, size=QB):
    return [(o, min(size, total - o)) for o in range(0, total, size)]


def _build_program(KP: int):
    from concourse import bacc, mybir
    from concourse.tile import TileContext

    JG = KP // P
    f32 = mybir.dt.float32
    bf16 = mybir.dt.bfloat16
    Exp = mybir.ActivationFunctionType.Exp
    nc = bacc.Bacc(None, target_bir_lowering=False)

    xT_d = nc.declare_dram_parameter("xT", [P, NQB, 6, QB], bf16, False)
    xTk_d = nc.declare_dram_parameter("xTk", [P, 6, KP], bf16, False)
    kfL_d = nc.declare_dram_parameter("kfL", [P, HPC, D], bf16, False)
    wkq_d = nc.declare_dram_parameter("wkqT", [P, 12, 192], bf16, False)
    wv_d = nc.declare_dram_parameter("wvT", [P, 6, 192], bf16, False)
    pT01_d = nc.declare_dram_parameter("pT01", [P, 6, P], bf16, False)
    pT2_d = nc.declare_dram_parameter("pT2pk", [P, 3, P], bf16, False)
    out_d = nc.declare_dram_parameter("outT", [P, NQB, 6, QB], bf16, True)

    NHALF = N // 2

    with TileContext(nc) as tc:
        with (
            tc.tile_pool(name="const", bufs=1) as cpool,
            tc.tile_pool(name="work", bufs=1) as wpool,
            tc.tile_pool(name="pt", bufs=14) as ptpool,
            tc.tile_pool(name="rb", bufs=3) as rbpool,
            tc.tile_pool(name="outp", bufs=2) as opool,
            tc.tile_pool(name="ps", bufs=2, space="PSUM") as pspool,
            tc.tile_pool(name="po", bufs=2, space="PSUM") as popool,
            tc.tile_pool(name="pp", bufs=1, space="PSUM") as pppool,
        ):
            # ---- PE warm-up: dummy matmuls with no data deps keep the
            # PE busy through the DMA phase so HAM un-throttles (K=8/8)
            # before the first real projection matmul.
            warm_i = cpool.tile([1, 8], f32)
            warm_o = cpool.tile([1, 8], bf16)
            warm_w = cpool.tile([P, P], bf16)
            nc.vector.memset(warm_i[:], 0.0)
            nc.vector.memset(warm_w[:], 0.0)
            pwarm = pspool.tile([P, 2, QB], f32, name="pwarm", tag="ps")
            for _ in range(40):
                nc.tensor.matmul(
                    pwarm[:, 0, 0:P], warm_w[:], warm_w[:], start=True, stop=True
                )

            # ---- constant tiles + input DMAs.
            # Scalar queue: small weight tensors only (done before first exp).
            # Sync/GpSimd queues: bulk x / xk traffic. xTk first (gates k/v
            # proj), then xT in 512-query quarters so the qb0 q-projection
            # unblocks as early as possible.
            wkq = cpool.tile([P, 12, 192], bf16)
            wv = cpool.tile([P, 6, 192], bf16)
            xT = cpool.tile([P, NQB, 6, QB], bf16)
            xTk = cpool.tile([P, 6, KP], bf16)
            pT01 = cpool.tile([P, 6, P], bf16)
            pT2 = cpool.tile([P, 3, P], bf16)

            # critical gates first: wk/wq on the scalar HWDGE ring, the two
            # xTk halves split across the sync/gpsimd rings, then the xT
            # quarters (one 3D DMA each), then everything else.
            nc.scalar.dma_start(wkq[:], wkq_d[:])
            nc.sync.dma_start(xTk[:, 0:3, :], xTk_d[:, 0:3, :])
            nc.gpsimd.dma_start(xT[:, 0, :, :], xT_d[:, 0, :, :])
            nc.scalar.dma_start(xTk[:, 3:6, :], xTk_d[:, 3:6, :])
            nc.sync.dma_start(xT[:, 1, :, :], xT_d[:, 1, :, :])
            nc.gpsimd.dma_start(xT[:, 2, :, :], xT_d[:, 2, :, :])
            nc.scalar.dma_start(wv[:], wv_d[:])
            nc.sync.dma_start(xT[:, 3, :, :], xT_d[:, 3, :, :])
            nc.gpsimd.dma_start(pT2[:], pT2_d[:])
            nc.sync.dma_start(pT01[:], pT01_d[:])
            # exp-table preload: after the scalar-ring DMA issues so the
            # ~2.7us ACT_TABLE_LOAD doesn't delay them
            nc.scalar.activation(warm_o[:], warm_i[:], Exp)

            # ---- work tiles
            qT01 = wpool.tile([P, N], bf16)
            qT2d = wpool.tile([P, N], bf16)   # head2 q, duplicated in halves
            kT01 = wpool.tile([P, KP], bf16)
            kT2d = wpool.tile([P, KP], bf16)  # head2 k, duplicated in halves
            v_sb = wpool.tile([P, JG, 2 * HPC, D], bf16)
            OT01 = wpool.tile([P, NQB, QB], bf16)
            OT2d = wpool.tile([P, NQB, QB], bf16)  # head2 O^T, duplicated

            # V indicator slots: 1.0 everywhere except the padded tail block,
            # which comes from a tiny DMA.
            if JG > 1:
                nc.gpsimd.memset(v_sb[:, 0 : JG - 1, 0 : 2 * HPC : 2, :], 1.0)
            nc.scalar.dma_start(v_sb[:, JG - 1, 0 : 2 * HPC : 2, :], kfL_d[:])

            kchunks = _chunks(KP)

            # ---- projection building blocks (each emits its own PSUM tile
            # allocs; emission order = scheduler priority).
            def k01_chunk(o, sz):
                ps = pspool.tile([P, 2, QB], f32, name="psq", tag="ps")
                for t in range(6):
                    nc.tensor.matmul(
                        ps[:, 0, 0:sz], wkq[:, t, 0:P], xTk[:, t, o : o + sz],
                        start=(t == 0), stop=(t == 5),
                    )
                nc.vector.tensor_copy(kT01[:, o : o + sz], ps[:, 0, 0:sz])

            def k2_pair(pair):
                ps = pspool.tile([P, 2, QB], f32, name="psq2", tag="ps")
                (o0, sz0) = pair[0]
                for t in range(6):
                    nc.tensor.matmul(
                        ps[0:D, 0, 0:sz0], wkq[:, t, P:192],
                        xTk[:, t, o0 : o0 + sz0],
                        start=(t == 0), stop=(t == 5),
                    )
                    if len(pair) == 2:
                        (o1, sz1) = pair[1]
                        nc.tensor.matmul(
                            ps[D:P, 1, 0:sz1], wkq[:, t, P:192],
                            xTk[:, t, o1 : o1 + sz1],
                            start=(t == 0), stop=(t == 5),
                        )
                nc.vector.tensor_copy(kT2d[0:D, o0 : o0 + sz0], ps[0:D, 0, 0:sz0])
                if len(pair) == 2:
                    nc.vector.tensor_copy(
                        kT2d[0:D, pair[1][0] : pair[1][0] + pair[1][1]],
                        ps[D:P, 1, 0 : pair[1][1]],
                    )

            def q01_chunk(ci):
                o = ci * QB
                ps = pspool.tile([P, 2, QB], f32, name="psq", tag="ps")
                for t in range(6):
                    nc.tensor.matmul(
                        ps[:, 0, :], wkq[:, 6 + t, 0:P], xT[:, ci, t, :],
                        start=(t == 0), stop=(t == 5),
                    )
                nc.vector.tensor_copy(qT01[:, o : o + QB], ps[:, 0, :])

            def q2_pair(c0, c1):
                # two query chunks packed in column halves, dup'd by DMA;
                # c1 may equal c0 (single chunk -> direct duplicate, no DMA)
                ps = pspool.tile([P, 2, QB], f32, name="psq2", tag="ps")
                o0, o1 = c0 * QB, c1 * QB
                for t in range(6):
                    nc.tensor.matmul(
                        ps[0:D, 0, :], wkq[:, 6 + t, P:192], xT[:, c0, t, :],
                        start=(t == 0), stop=(t == 5),
                    )
                    if c1 != c0:
                        nc.tensor.matmul(
                            ps[D:P, 1, :], wkq[:, 6 + t, P:192], xT[:, c1, t, :],
                            start=(t == 0), stop=(t == 5),
                        )
                if c1 == c0:
                    nc.vector.tensor_copy(qT2d[0:D, o0 : o0 + QB], ps[0:D, 0, :])
                    nc.vector.tensor_copy(qT2d[D:P, o0 : o0 + QB], ps[0:D, 0, :])
                else:
                    nc.vector.tensor_copy(qT2d[0:D, o0 : o0 + QB], ps[0:D, 0, :])
                    nc.vector.tensor_copy(qT2d[0:D, o1 : o1 + QB], ps[D:P, 1, :])
                    nc.vector.tensor_copy(
                        qT2d[D:P, o0 : o1 + QB], qT2d[0:D, o0 : o1 + QB]
                    )

            def v_pair(i):
                blocks = [j for j in (i, i + 1) if j < JG]
                ps = pspool.tile([P, 2, QB], f32, name="psv", tag="ps")
                for s, j in enumerate(blocks):
                    for t in range(6):
                        nc.tensor.matmul(
                            ps[:, s, 0:192],
                            xTk[:, t, j * P : (j + 1) * P],
                            wv[:, t, :],
                            start=(t == 0), stop=(t == 5),
                        )
                for s, j in enumerate(blocks):
                    nc.vector.tensor_copy(
                        v_sb[:, j, 1 : 2 * HPC : 2, :], ps[:, s, 0:192]
                    )

            # ---- attention primitives: one "slot" = one score-pair +
            # one exp; the engine queues are static FIFO, so the emission
            # order below IS the runtime schedule. Per slot we emit the AV
            # matmuls of slot s-2 (whose exp has completed by then), the
            # scores of slot s, and ~500ns of budgeted filler work.
            pts = {}
            po_t = {}
            budget = [0.0]

            KC_TAGS = ["kc%d" % c for c in range(1, len(kchunks))]

            def emit_sc(key):
                qb, kind, i = key
                if kind == "a":
                    if qb >= 1:
                        need("q01_%d" % qb)
                    if i >= 4:
                        for t_ in KC_TAGS[: (i * P) // QB]:
                            need(t_)
                else:
                    need("k2")
                    need("q2_0" if qb <= 1 else "q2_%d" % qb)
                qs = slice(qb * QB, (qb + 1) * QB)
                ps = pspool.tile([P, 2, QB], f32, name="psc", tag="ps")
                pt = ptpool.tile([P, 2, QB], bf16, name="pt", tag="pt")
                if kind == "a":
                    jg = i
                    nc.tensor.matmul(
                        ps[:, 0, :], kT01[0:D, jg * P : (jg + 1) * P],
                        qT01[0:D, qs], start=True, stop=True,
                    )
                    nc.tensor.matmul(
                        ps[:, 1, :], kT01[D:P, jg * P : (jg + 1) * P],
                        qT01[D:P, qs], start=True, stop=True,
                    )
                    nc.scalar.activation(pt[:], ps[:], Exp, scale=float(SCALE))
                else:
                    j0, j1 = 2 * i, 2 * i + 1
                    hasb = j1 < JG
                    nc.tensor.matmul(
                        ps[:, 0, :], kT2d[0:D, j0 * P : (j0 + 1) * P],
                        qT2d[0:D, qs], start=True, stop=True,
                    )
                    if hasb:
                        nc.tensor.matmul(
                            ps[:, 1, :], kT2d[D:P, j1 * P : (j1 + 1) * P],
                            qT2d[D:P, qs], start=True, stop=True,
                        )
                        nc.scalar.activation(pt[:], ps[:], Exp, scale=float(SCALE))
                    else:
                        nc.scalar.activation(
                            pt[:, 0, :], ps[:, 0, :], Exp, scale=float(SCALE)
                        )
                pts[key] = pt

            NT2 = (JG + 1) // 2

            def emit_av(key):
                qb, kind, i = key
                if kind == "a":
                    need("v%d" % i)
                else:
                    need("v%d" % min(2 * i + 1, JG - 1))
                pt = pts.pop(key)
                if kind == "a":
                    if i == 0:
                        po_t[(qb, 0)] = popool.tile([P, QB], f32, name="po", tag="po")
                        po_t[(qb, 1)] = popool.tile([P, QB], f32, name="po", tag="po")
                    po0, po1 = po_t[(qb, 0)], po_t[(qb, 1)]
                    nc.tensor.matmul(
                        po0[:], v_sb[:, i, 0:2, :], pt[:, 0, :],
                        start=(i == 0), stop=(i == JG - 1),
                    )
                    nc.tensor.matmul(
                        po1[:], v_sb[:, i, 2:4, :], pt[:, 1, :],
                        start=(i == 0), stop=(i == JG - 1),
                    )
                    if i == JG - 1:
                        rb0 = rbpool.tile([D, QB], f32, tag="rb")
                        nc.vector.reciprocal_approx_fast(rb0[:], po0[0:D, :])
                        nc.vector.tensor_mul(OT01[0:D, qb, :], po0[D:P, :], rb0[:])
                        rb1 = rbpool.tile([D, QB], f32, tag="rb")
                        nc.vector.reciprocal_approx_fast(rb1[:], po1[0:D, :])
                        nc.vector.tensor_mul(OT01[D:P, qb, :], po1[D:P, :], rb1[:])
                else:
                    if i == 0:
                        po_t[(qb, 2)] = popool.tile([P, QB], f32, name="po", tag="po")
                    po2 = po_t[(qb, 2)]
                    j0, j1 = 2 * i, 2 * i + 1
                    hasb = j1 < JG
                    nc.tensor.matmul(
                        po2[:], v_sb[:, j0, 4:6, :], pt[:, 0, :],
                        start=(i == 0), stop=(i == NT2 - 1 and not hasb),
                    )
                    if hasb:
                        nc.tensor.matmul(
                            po2[:], v_sb[:, j1, 4:6, :], pt[:, 1, :],
                            start=False, stop=(i == NT2 - 1),
                        )
                    if i == NT2 - 1:
                        rb2 = rbpool.tile([D, QB], f32, tag="rb")
                        nc.vector.reciprocal_approx_fast(rb2[:], po2[0:D, :])
                        nc.vector.tensor_mul(OT2d[0:D, qb, :], po2[D:P, :], rb2[:])
                        nc.vector.tensor_mul(OT2d[D:P, qb, :], po2[D:P, :], rb2[:])
                        # projection for this query block becomes filler work
                        for j in range(3):
                            filler_q.insert(
                                j,
                                (800, "p%d_%d" % (qb, j),
                                 _mk_proj_unit(qb, j, last=(qb == 3))),
                            )

            ob_t = {}

            def _mk_proj_unit(qb, j, last=False):
                def emit():
                    if j == 0:
                        ob_t[qb] = opool.tile([P, 6, QB], bf16, name="ob", tag="ob")
                    ob = ob_t[qb]
                    # the final block's projection runs after the last exp:
                    # the score pool (4 banks) is free then, so use it to
                    # pipeline the three units instead of serializing on pp
                    if last:
                        pp = pspool.tile([P, 2, QB], f32, name="pp", tag="ps")
                    else:
                        pp = pppool.tile([P, 2, QB], f32, name="pp", tag="pp")
                    nc.tensor.matmul(
                        pp[:, 0, :], pT01[:, 2 * j, :], OT01[:, qb, :],
                        start=True, stop=False,
                    )
                    nc.tensor.matmul(
                        pp[:, 1, :], pT01[:, 2 * j + 1, :], OT01[:, qb, :],
                        start=True, stop=False,
                    )
                    nc.tensor.matmul(
                        pp[:, 0, :], pT2[0:D, j, :], OT2d[0:D, qb, :],
                        start=False, stop=True,
                    )
                    nc.tensor.matmul(
                        pp[:, 1, :], pT2[D:P, j, :], OT2d[D:P, qb, :],
                        start=False, stop=True,
                    )
                    if last and j != 1:
                        nc.scalar.copy(ob[:, 2 * j : 2 * j + 2, :], pp[:])
                    else:
                        nc.vector.tensor_copy(ob[:, 2 * j : 2 * j + 2, :], pp[:])
                    outq = nc.sync if qb % 2 == 0 else nc.gpsimd
                    outq.dma_start(
                        out_d[:, qb, 2 * j : 2 * j + 2, :],
                        ob[:, 2 * j : 2 * j + 2, :],
                    )
                return emit

            def v_block(j):
                def emit():
                    ps = pspool.tile([P, 2, QB], f32, name="psv", tag="ps")
                    for t in range(6):
                        nc.tensor.matmul(
                            ps[:, 0, 0:192],
                            xTk[:, t, j * P : (j + 1) * P],
                            wv[:, t, :],
                            start=(t == 0), stop=(t == 5),
                        )
                    nc.vector.tensor_copy(
                        v_sb[:, j, 1 : 2 * HPC : 2, :], ps[:, 0, 0:192]
                    )
                return emit

            def k2_all():
                for i in range(0, len(kchunks), 2):
                    k2_pair(kchunks[i : i + 2])
                nc.vector.tensor_copy(kT2d[D:P, :], kT2d[0:D, :])

            # ---- the static schedule.
            # stream block order: h01 of qb0..2, then their h2 blocks, then
            # qb3 -- this pushes the kT2d/qT2d deadline ~20us out so the
            # k2/q2 filler projections fit in the early slots.
            SL = []
            for qb, kind in (
                (0, "a"), (1, "a"), (2, "a"), (3, "a"),
                (0, "b"), (1, "b"), (2, "b"), (3, "b"),
            ):
                n = JG if kind == "a" else NT2
                for i in range(n):
                    SL.append((qb, kind, i))

            filler_q = []
            filler_q.append((660, "v0", v_block(0)))
            filler_q.append((660, "v1", v_block(1)))
            for ci, (o, sz) in enumerate(kchunks[1:]):
                filler_q.append(
                    (1400, "kc%d" % (ci + 1), lambda o=o, sz=sz: k01_chunk(o, sz))
                )
            for j in (2, 3, 4, 5):
                filler_q.append((660, "v%d" % j, v_block(j)))
            filler_q.append((1400, "q01_1", lambda: q01_chunk(1)))
            for j in range(6, JG):
                filler_q.append((660, "v%d" % j, v_block(j)))
            filler_q.append((2400, "k2", k2_all))
            filler_q.append((2400, "q2_0", lambda: q2_pair(0, 1)))
            filler_q.append((1400, "q01_2", lambda: q01_chunk(2)))
            filler_q.append((2800, "q2_2", lambda: q2_pair(2, 2)))
            filler_q.append((1400, "q01_3", lambda: q01_chunk(3)))
            filler_q.append((2800, "q2_3", lambda: q2_pair(3, 3)))
            _done_tags = set()

            def need(tag):
                # force-emit queued fillers (in order) until `tag` has run;
                # guarantees producers are emitted before their readers.
                while tag not in _done_tags and filler_q:
                    c, t, fn = filler_q.pop(0)
                    fn()
                    _done_tags.add(t)
                    budget[0] -= c

            # prelude: the critical chain to the first exp. kc0's t3-5
            # matmuls wait the second xTk half, so the q01(0) matmuls are
            # interleaved between (they only need xT quarter 0 + wq).
            ps_k0 = pspool.tile([P, 2, QB], f32, name="ps_k0", tag="ps")
            ps_q0 = pspool.tile([P, 2, QB], f32, name="ps_q0", tag="ps")
            o0, sz0 = kchunks[0]
            for t in range(3):
                nc.tensor.matmul(
                    ps_k0[:, 0, 0:sz0], wkq[:, t, 0:P], xTk[:, t, o0 : o0 + sz0],
                    start=(t == 0), stop=False,
                )
            for t in range(6):
                nc.tensor.matmul(
                    ps_q0[:, 0, :], wkq[:, 6 + t, 0:P], xT[:, 0, t, :],
                    start=(t == 0), stop=(t == 5),
                )
            for t in range(3, 6):
                nc.tensor.matmul(
                    ps_k0[:, 0, 0:sz0], wkq[:, t, 0:P], xTk[:, t, o0 : o0 + sz0],
                    start=False, stop=(t == 5),
                )
            nc.vector.tensor_copy(qT01[:, 0:QB], ps_q0[:, 0, :])
            nc.vector.tensor_copy(kT01[:, o0 : o0 + sz0], ps_k0[:, 0, 0:sz0])

            def pump(extra=0.0):
                budget[0] += extra
                while filler_q and budget[0] >= filler_q[0][0] - 1400:
                    c, t, fn = filler_q.pop(0)
                    fn()
                    _done_tags.add(t)
                    budget[0] -= c

            for s in range(len(SL) + 2):
                if s >= 2:
                    emit_av(SL[s - 2])
                if s < len(SL):
                    emit_sc(SL[s])
                pump(500.0)

            # tail: flush remaining fillers (the last projections)
            while filler_q:
                _, _t, fn = filler_q.pop(0)
                fn()

    nc.finalize()
    return nc


def _prep_inputs(x, mask, qkv_w, proj_w):
    """Build the 8 per-core input maps. Returns (in_maps, KP)."""
    idx = [np.nonzero(mask[b] == 0.0)[0] for b in range(B)]
    nk = max(len(i) for i in idx)
    KP = max(P, int(math.ceil(nk / P)) * P)
    JG = KP // P

    per_batch = []
    for b in range(B):
        xTb = np.ascontiguousarray(x[b].T)  # [C, N] f32
        xT_in = np.ascontiguousarray(
            xTb.reshape(6, P, NQB, QB).transpose(1, 2, 0, 3)
        ).astype(BF16)  # [P, NQB, 6, QB] -- contiguous 6KB line per quarter
        xk = np.zeros((C, KP), np.float32)
        xk[:, : len(idx[b])] = xTb[:, idx[b]]
        xTk_in = xk.reshape(6, P, KP).transpose(1, 0, 2).astype(BF16)
        kfv = np.zeros((KP,), np.float32)
        kfv[: len(idx[b])] = 1.0
        kfL_in = np.ascontiguousarray(
            np.broadcast_to(
                kfv[(JG - 1) * P :][:, None, None], (P, HPC, D)
            )
        ).astype(BF16)
        per_batch.append((xT_in, xTk_in, kfL_in))

    in_maps = []
    for c in range(NCORES):
        b, g = c // 4, c % 4
        h0 = HPC * g
        xT_in, xTk_in, kfL_in = per_batch[b]
        m = {"xT": xT_in, "xTk": xTk_in, "kfL": kfL_in}
        ws = {}
        for name, off in (("wqT", 0), ("wkT", C), ("wvT", 2 * C)):
            w = qkv_w[off + h0 * D : off + (h0 + HPC) * D]  # [192, C]
            ws[name] = (
                np.ascontiguousarray(w.T).reshape(6, P, 192).transpose(1, 0, 2).astype(BF16)
            )
        m["wkqT"] = np.ascontiguousarray(
            np.concatenate([ws["wkT"], ws["wqT"]], axis=1)
        )
        m["wvT"] = ws["wvT"]
        pw = proj_w[:, h0 * D : h0 * D + HPC * D]  # [768, 192]
        m["pT01"] = np.ascontiguousarray(pw[:, :P].T).reshape(P, 6, P).astype(BF16)
        pT2o = np.ascontiguousarray(pw[:, P:].T).reshape(D, 6, P)  # [64, 6, 128]
        pT2pk = np.empty((P, 3, P), np.float32)
        for j in range(3):
            pT2pk[0:D, j] = pT2o[:, 2 * j]
            pT2pk[D:P, j] = pT2o[:, 2 * j + 1]
        m["pT2pk"] = pT2pk.astype(BF16)
        in_maps.append(m)
    return in_maps, KP


_CACHE = {}


def _get_program(KP):
    if KP not in _CACHE:
        _CACHE[KP] = _build_program(KP)
    return _CACHE[KP]


def _gather_output(results, proj_b):
    out = np.empty((B, N, C), np.float32)
    for b in range(B):
        acc = None
        for c in range(4 * b, 4 * b + 4):
            a = results[c]["outT"]  # [128, NQB, 6, QB] bf16
            a = np.asarray(a, np.float32).transpose(2, 0, 1, 3).reshape(C, N)
            acc = a if acc is None else acc + a
        out[b] = acc.T + proj_b[None, :]
    return out


def kernel(x, mask, qkv_w, proj_w, proj_b, _want_results=False):
    from concourse.bass_utils import run_bass_kernel_spmd

    x = np.asarray(x, np.float32)
    mask = np.asarray(mask, np.float32)
    qkv_w = np.asarray(qkv_w, np.float32)
    proj_w = np.asarray(proj_w, np.float32)
    proj_b = np.asarray(proj_b, np.float32)

    in_maps, KP = _prep_inputs(x, mask, qkv_w, proj_w)
    nc = _get_program(KP)
    res = run_bass_kernel_spmd(nc, in_maps, list(range(NCORES)))

    out = _gather_output(res.results, proj_b)
    if _want_results:
        return out, res
    return out


# revision 19
# speedup vs baseline: 1.0681x; 1.0681x over previous
"""Masked multi-head attention kernel for 8 Trainium2 NeuronCores.

Strategy (v2 — PE-packing rework of the v1 baseline):
  - 24 (batch, head) pairs sharded as: core c -> batch c//4, heads [3*(c%4) .. 3*(c%4)+2].
  - Key-padding mask handled by HOST-side gather: only unmasked key positions are
    shipped/computed. Padded key slots get zeroed K columns (scores=0 -> exp=1)
    and a 0 in the indicator slot of V, so they contribute nothing.
  - Softmax without max-subtraction (scores ~ N(0,1); masked keys excluded).
  - Row-sum of exp folded into the AV matmul via an indicator slot on V.
  - PE tile packing (the big v2 win): all D=64-contraction matmuls run as
    concurrent PAIRS in the two 64-row halves of the PE array
    (tile_position auto-derived from partition bases):
      * scores: head0 (partitions 0-63) paired with head1 (64-127) per key
        block; head2 paired with itself via duplicated kT2/qT2 (dup'd by
        SBUF->SBUF DMA, free on the DMA engines).
      * out-proj: the K=64 tail (head2 dims) packed as column-group pairs
        (even cg rows 0-63, odd cg rows 64-127) against duplicated OT2.
      * qT2/kT2 projections: M=64 outputs packed 2 query-chunks per pass
        via column tiling.
  - One exp per score pair: [128, 2*512] ACTIVATE (amortizes ACT overhead,
    covers both heads of the pair).
  - ACT table preloaded by a dummy exp at t=0; all DMAs issued on the
    Sync + GpSimd queues so the Scalar engine does nothing but exp.
  - V indicator slots built on-device (gpsimd memset) + a tiny DMA for the
    padded tail block.
  - PSUM: 4 banks score pairs (x2), 2 banks AV accumulators, 2 banks proj.
  - bf16 matmul inputs, fp32 PSUM accumulation, bf16 output partials
    (host sums the 4 partials per batch in fp32, adds proj_b).
"""

import math

import numpy as np
import ml_dtypes

BF16 = ml_dtypes.bfloat16
B, N, C = 2, 2048, 768
H = 12
D = 64
HPC = 3          # heads per core
P = 128
QB = 512         # query block
NQB = N // QB
SCALE = D ** -0.5
NCORES = 8


def _chunks(total, size=QB):
    return [(o, min(size, total - o)) for o in range(0, total, size)]


def _build_program(KP: int):
    from concourse import bacc, mybir
    from concourse.tile import TileContext

    JG = KP // P
    f32 = mybir.dt.float32
    bf16 = mybir.dt.bfloat16
    Exp = mybir.ActivationFunctionType.Exp
    nc = bacc.Bacc(None, target_bir_lowering=False)

    xT_d = nc.declare_dram_parameter("xT", [P, NQB, 6, QB], bf16, False)
    xTk_d = nc.declare_dram_parameter("xTk", [P, 6, KP], bf16, False)
    kfL_d = nc.declare_dram_parameter("kfL", [P, HPC, D], bf16, False)
    wkq_d = nc.declare_dram_parameter("wkqT", [P, 12, 192], bf16, False)
    wv_d = nc.declare_dram_parameter("wvT", [P, 6, 192], bf16, False)
    pT01_d = nc.declare_dram_parameter("pT01", [P, 6, P], bf16, False)
    pT2_d = nc.declare_dram_parameter("pT2pk", [P, 3, P], bf16, False)
    out_d = nc.declare_dram_parameter("outT", [P, NQB, 6, QB], bf16, True)

    NHALF = N // 2

    with TileContext(nc) as tc:
        with (
            tc.tile_pool(name="const", bufs=1) as cpool,
            tc.tile_pool(name="work", bufs=1) as wpool,
            tc.tile_pool(name="pt", bufs=14) as ptpool,
            tc.tile_pool(name="rb", bufs=3) as rbpool,
            tc.tile_pool(name="outp", bufs=2) as opool,
            tc.tile_pool(name="ps", bufs=2, space="PSUM") as pspool,
            tc.tile_pool(name="po", bufs=2, space="PSUM") as popool,
            tc.tile_pool(name="pp", bufs=1, space="PSUM") as pppool,
        ):
            # ---- PE warm-up: dummy matmuls with no data deps keep the
            # PE busy through the DMA phase so HAM un-throttles (K=8/8)
            # before the first real projection matmul.
            warm_i = cpool.tile([1, 8], f32)
            warm_o = cpool.tile([1, 8], bf16)
            nc.vector.memset(warm_i[:], 0.0)

            # ---- constant tiles + input DMAs.
            # Scalar queue: small weight tensors only (done before first exp).
            # Sync/GpSimd queues: bulk x / xk traffic. xTk first (gates k/v
            # proj), then xT in 512-query quarters so the qb0 q-projection
            # unblocks as early as possible.
            wkq = cpool.tile([P, 12, 192], bf16)
            wv = cpool.tile([P, 6, 192], bf16)
            xT = cpool.tile([P, NQB, 6, QB], bf16)
            xTk = cpool.tile([P, 6, KP], bf16)
            pT01 = cpool.tile([P, 6, P], bf16)
            pT2 = cpool.tile([P, 3, P], bf16)

            # critical gates first: wk/wq on the scalar HWDGE ring, the two
            # xTk halves split across the sync/gpsimd rings, then the xT
            # quarters (one 3D DMA each), then everything else.
            nc.scalar.dma_start(wkq[:], wkq_d[:])
            nc.sync.dma_start(xTk[:, 0:3, :], xTk_d[:, 0:3, :])
            nc.gpsimd.dma_start(xT[:, 0, :, :], xT_d[:, 0, :, :])
            nc.scalar.dma_start(xTk[:, 3:6, :], xTk_d[:, 3:6, :])
            nc.sync.dma_start(xT[:, 1, :, :], xT_d[:, 1, :, :])
            nc.gpsimd.dma_start(xT[:, 2, :, :], xT_d[:, 2, :, :])
            nc.scalar.dma_start(wv[:], wv_d[:])
            nc.sync.dma_start(xT[:, 3, :, :], xT_d[:, 3, :, :])
            nc.gpsimd.dma_start(pT2[:], pT2_d[:])
            nc.sync.dma_start(pT01[:], pT01_d[:])
            # exp-table preload: after the scalar-ring DMA issues so the
            # ~2.7us ACT_TABLE_LOAD doesn't delay them
            nc.scalar.activation(warm_o[:], warm_i[:], Exp)

            # ---- work tiles
            qT01 = wpool.tile([P, N], bf16)
            qT2d = wpool.tile([P, N], bf16)   # head2 q, duplicated in halves
            kT01 = wpool.tile([P, KP], bf16)
            kT2d = wpool.tile([P, KP], bf16)  # head2 k, duplicated in halves
            v_sb = wpool.tile([P, JG, 2 * HPC, D], bf16)
            OT01 = wpool.tile([P, NQB, QB], bf16)
            OT2d = wpool.tile([P, NQB, QB], bf16)  # head2 O^T, duplicated

            # V indicator slots: 1.0 everywhere except the padded tail block,
            # which comes from a tiny DMA.
            if JG > 1:
                nc.gpsimd.memset(v_sb[:, 0 : JG - 1, 0 : 2 * HPC : 2, :], 1.0)
            nc.scalar.dma_start(v_sb[:, JG - 1, 0 : 2 * HPC : 2, :], kfL_d[:])

            kchunks = _chunks(KP)

            # ---- projection building blocks (each emits its own PSUM tile
            # allocs; emission order = scheduler priority).
            def k01_chunk(o, sz):
                ps = pspool.tile([P, 2, QB], f32, name="psq", tag="ps")
                for t in range(6):
                    nc.tensor.matmul(
                        ps[:, 0, 0:sz], wkq[:, t, 0:P], xTk[:, t, o : o + sz],
                        start=(t == 0), stop=(t == 5),
                    )
                nc.vector.tensor_copy(kT01[:, o : o + sz], ps[:, 0, 0:sz])

            def k2_pair(pair):
                ps = pspool.tile([P, 2, QB], f32, name="psq2", tag="ps")
                (o0, sz0) = pair[0]
                for t in range(6):
                    nc.tensor.matmul(
                        ps[0:D, 0, 0:sz0], wkq[:, t, P:192],
                        xTk[:, t, o0 : o0 + sz0],
                        start=(t == 0), stop=(t == 5),
                    )
                    if len(pair) == 2:
                        (o1, sz1) = pair[1]
                        nc.tensor.matmul(
                            ps[D:P, 1, 0:sz1], wkq[:, t, P:192],
                            xTk[:, t, o1 : o1 + sz1],
                            start=(t == 0), stop=(t == 5),
                        )
                nc.vector.tensor_copy(kT2d[0:D, o0 : o0 + sz0], ps[0:D, 0, 0:sz0])
                if len(pair) == 2:
                    nc.vector.tensor_copy(
                        kT2d[0:D, pair[1][0] : pair[1][0] + pair[1][1]],
                        ps[D:P, 1, 0 : pair[1][1]],
                    )

            def q01_chunk(ci):
                o = ci * QB
                ps = pspool.tile([P, 2, QB], f32, name="psq", tag="ps")
                for t in range(6):
                    nc.tensor.matmul(
                        ps[:, 0, :], wkq[:, 6 + t, 0:P], xT[:, ci, t, :],
                        start=(t == 0), stop=(t == 5),
                    )
                nc.vector.tensor_copy(qT01[:, o : o + QB], ps[:, 0, :])

            def q2_pair(c0, c1):
                # two query chunks packed in column halves, dup'd by DMA;
                # c1 may equal c0 (single chunk -> direct duplicate, no DMA)
                ps = pspool.tile([P, 2, QB], f32, name="psq2", tag="ps")
                o0, o1 = c0 * QB, c1 * QB
                for t in range(6):
                    nc.tensor.matmul(
                        ps[0:D, 0, :], wkq[:, 6 + t, P:192], xT[:, c0, t, :],
                        start=(t == 0), stop=(t == 5),
                    )
                    if c1 != c0:
                        nc.tensor.matmul(
                            ps[D:P, 1, :], wkq[:, 6 + t, P:192], xT[:, c1, t, :],
                            start=(t == 0), stop=(t == 5),
                        )
                if c1 == c0:
                    nc.vector.tensor_copy(qT2d[0:D, o0 : o0 + QB], ps[0:D, 0, :])
                    nc.vector.tensor_copy(qT2d[D:P, o0 : o0 + QB], ps[0:D, 0, :])
                else:
                    nc.vector.tensor_copy(qT2d[0:D, o0 : o0 + QB], ps[0:D, 0, :])
                    nc.vector.tensor_copy(qT2d[0:D, o1 : o1 + QB], ps[D:P, 1, :])
                    nc.vector.tensor_copy(
                        qT2d[D:P, o0 : o1 + QB], qT2d[0:D, o0 : o1 + QB]
                    )

            def v_pair(i):
                blocks = [j for j in (i, i + 1) if j < JG]
                ps = pspool.tile([P, 2, QB], f32, name="psv", tag="ps")
                for s, j in enumerate(blocks):
                    for t in range(6):
                        nc.tensor.matmul(
                            ps[:, s, 0:192],
                            xTk[:, t, j * P : (j + 1) * P],
                            wv[:, t, :],
                            start=(t == 0), stop=(t == 5),
                        )
                for s, j in enumerate(blocks):
                    nc.vector.tensor_copy(
                        v_sb[:, j, 1 : 2 * HPC : 2, :], ps[:, s, 0:192]
                    )

            # ---- attention primitives: one "slot" = one score-pair +
            # one exp; the engine queues are static FIFO, so the emission
            # order below IS the runtime schedule. Per slot we emit the AV
            # matmuls of slot s-2 (whose exp has completed by then), the
            # scores of slot s, and ~500ns of budgeted filler work.
            pts = {}
            po_t = {}
            budget = [0.0]

            KC_TAGS = ["kc%d" % c for c in range(1, len(kchunks))]

            def emit_sc(key):
                qb, kind, i = key
                if kind == "a":
                    if qb >= 1:
                        need("q01_%d" % qb)
                    if i >= 4:
                        for t_ in KC_TAGS[: (i * P) // QB]:
                            need(t_)
                else:
                    need("k2")
                    need("q2_0" if qb <= 1 else "q2_%d" % qb)
                qs = slice(qb * QB, (qb + 1) * QB)
                ps = pspool.tile([P, 2, QB], f32, name="psc", tag="ps")
                pt = ptpool.tile([P, 2, QB], bf16, name="pt", tag="pt")
                if kind == "a":
                    jg = i
                    nc.tensor.matmul(
                        ps[:, 0, :], kT01[0:D, jg * P : (jg + 1) * P],
                        qT01[0:D, qs], start=True, stop=True,
                    )
                    nc.tensor.matmul(
                        ps[:, 1, :], kT01[D:P, jg * P : (jg + 1) * P],
                        qT01[D:P, qs], start=True, stop=True,
                    )
                    nc.scalar.activation(pt[:], ps[:], Exp, scale=float(SCALE))
                else:
                    j0, j1 = 2 * i, 2 * i + 1
                    hasb = j1 < JG
                    nc.tensor.matmul(
                        ps[:, 0, :], kT2d[0:D, j0 * P : (j0 + 1) * P],
                        qT2d[0:D, qs], start=True, stop=True,
                    )
                    if hasb:
                        nc.tensor.matmul(
                            ps[:, 1, :], kT2d[D:P, j1 * P : (j1 + 1) * P],
                            qT2d[D:P, qs], start=True, stop=True,
                        )
                        nc.scalar.activation(pt[:], ps[:], Exp, scale=float(SCALE))
                    else:
                        nc.scalar.activation(
                            pt[:, 0, :], ps[:, 0, :], Exp, scale=float(SCALE)
                        )
                pts[key] = pt

            NT2 = (JG + 1) // 2

            def emit_av(key):
                qb, kind, i = key
                if kind == "a":
                    need("v%d" % i)
                else:
                    need("v%d" % min(2 * i + 1, JG - 1))
                pt = pts.pop(key)
                if kind == "a":
                    if i == 0:
                        po_t[(qb, 0)] = popool.tile([P, QB], f32, name="po", tag="po")
                        po_t[(qb, 1)] = popool.tile([P, QB], f32, name="po", tag="po")
                    po0, po1 = po_t[(qb, 0)], po_t[(qb, 1)]
                    nc.tensor.matmul(
                        po0[:], v_sb[:, i, 0:2, :], pt[:, 0, :],
                        start=(i == 0), stop=(i == JG - 1),
                    )
                    nc.tensor.matmul(
                        po1[:], v_sb[:, i, 2:4, :], pt[:, 1, :],
                        start=(i == 0), stop=(i == JG - 1),
                    )
                    if i == JG - 1:
                        rb0 = rbpool.tile([D, QB], f32, tag="rb")
                        nc.vector.reciprocal_approx_fast(rb0[:], po0[0:D, :])
                        nc.vector.tensor_mul(OT01[0:D, qb, :], po0[D:P, :], rb0[:])
                        rb1 = rbpool.tile([D, QB], f32, tag="rb")
                        nc.vector.reciprocal_approx_fast(rb1[:], po1[0:D, :])
                        nc.vector.tensor_mul(OT01[D:P, qb, :], po1[D:P, :], rb1[:])
                else:
                    if i == 0:
                        po_t[(qb, 2)] = popool.tile([P, QB], f32, name="po", tag="po")
                    po2 = po_t[(qb, 2)]
                    j0, j1 = 2 * i, 2 * i + 1
                    hasb = j1 < JG
                    nc.tensor.matmul(
                        po2[:], v_sb[:, j0, 4:6, :], pt[:, 0, :],
                        start=(i == 0), stop=(i == NT2 - 1 and not hasb),
                    )
                    if hasb:
                        nc.tensor.matmul(
                            po2[:], v_sb[:, j1, 4:6, :], pt[:, 1, :],
                            start=False, stop=(i == NT2 - 1),
                        )
                    if i == NT2 - 1:
                        rb2 = rbpool.tile([D, QB], f32, tag="rb")
                        nc.vector.reciprocal_approx_fast(rb2[:], po2[0:D, :])
                        nc.vector.tensor_mul(OT2d[0:D, qb, :], po2[D:P, :], rb2[:])
                        nc.vector.tensor_mul(OT2d[D:P, qb, :], po2[D:P, :], rb2[:])
                        # projection for this query block becomes filler work
                        for j in range(3):
                            filler_q.insert(
                                j,
                                (800, "p%d_%d" % (qb, j),
                                 _mk_proj_unit(qb, j, last=(qb == 3))),
                            )

            ob_t = {}

            def _mk_proj_unit(qb, j, last=False):
                def emit():
                    if j == 0:
                        ob_t[qb] = opool.tile([P, 6, QB], bf16, name="ob", tag="ob")
                    ob = ob_t[qb]
                    # the final block's projection runs after the last exp:
                    # the score pool (4 banks) is free then, so use it to
                    # pipeline the three units instead of serializing on pp
                    if last:
                        pp = pspool.tile([P, 2, QB], f32, name="pp", tag="ps")
                    else:
                        pp = pppool.tile([P, 2, QB], f32, name="pp", tag="pp")
                    nc.tensor.matmul(
                        pp[:, 0, :], pT01[:, 2 * j, :], OT01[:, qb, :],
                        start=True, stop=False,
                    )
                    nc.tensor.matmul(
                        pp[:, 1, :], pT01[:, 2 * j + 1, :], OT01[:, qb, :],
                        start=True, stop=False,
                    )
                    nc.tensor.matmul(
                        pp[:, 0, :], pT2[0:D, j, :], OT2d[0:D, qb, :],
                        start=False, stop=True,
                    )
                    nc.tensor.matmul(
                        pp[:, 1, :], pT2[D:P, j, :], OT2d[D:P, qb, :],
                        start=False, stop=True,
                    )
                    if last and j != 1:
                        nc.scalar.copy(ob[:, 2 * j : 2 * j + 2, :], pp[:])
                    else:
                        nc.vector.tensor_copy(ob[:, 2 * j : 2 * j + 2, :], pp[:])
                    outq = nc.sync if qb % 2 == 0 else nc.gpsimd
                    outq.dma_start(
                        out_d[:, qb, 2 * j : 2 * j + 2, :],
                        ob[:, 2 * j : 2 * j + 2, :],
                    )
                return emit

            def v_block(j):
                def emit():
                    ps = pspool.tile([P, 2, QB], f32, name="psv", tag="ps")
                    for t in range(6):
                        nc.tensor.matmul(
                            ps[:, 0, 0:192],
                            xTk[:, t, j * P : (j + 1) * P],
                            wv[:, t, :],
                            start=(t == 0), stop=(t == 5),
                        )
                    nc.vector.tensor_copy(
                        v_sb[:, j, 1 : 2 * HPC : 2, :], ps[:, 0, 0:192]
                    )
                return emit

            def k2_all():
                for i in range(0, len(kchunks), 2):
                    k2_pair(kchunks[i : i + 2])
                nc.vector.tensor_copy(kT2d[D:P, :], kT2d[0:D, :])

            # ---- the static schedule.
            # stream block order: h01 of qb0..2, then their h2 blocks, then
            # qb3 -- this pushes the kT2d/qT2d deadline ~20us out so the
            # k2/q2 filler projections fit in the early slots.
            SL = []
            for qb, kind in (
                (0, "a"), (1, "a"), (2, "a"), (3, "a"),
                (0, "b"), (1, "b"), (2, "b"), (3, "b"),
            ):
                n = JG if kind == "a" else NT2
                for i in range(n):
                    SL.append((qb, kind, i))

            filler_q = []
            filler_q.append((660, "v0", v_block(0)))
            filler_q.append((660, "v1", v_block(1)))
            for ci, (o, sz) in enumerate(kchunks[1:]):
                filler_q.append(
                    (1400, "kc%d" % (ci + 1), lambda o=o, sz=sz: k01_chunk(o, sz))
                )
            for j in (2, 3, 4, 5):
                filler_q.append((660, "v%d" % j, v_block(j)))
            filler_q.append((1400, "q01_1", lambda: q01_chunk(1)))
            for j in range(6, JG):
                filler_q.append((660, "v%d" % j, v_block(j)))
            filler_q.append((2400, "k2", k2_all))
            filler_q.append((2400, "q2_0", lambda: q2_pair(0, 1)))
            filler_q.append((1400, "q01_2", lambda: q01_chunk(2)))
            filler_q.append((2800, "q2_2", lambda: q2_pair(2, 2)))
            filler_q.append((1400, "q01_3", lambda: q01_chunk(3)))
            filler_q.append((2800, "q2_3", lambda: q2_pair(3, 3)))
            _done_tags = set()

            def need(tag):
                # force-emit queued fillers (in order) until `tag` has run;
                # guarantees producers are emitted before their readers.
                # Does NOT charge the budget: a forced filler is already
                # late, and charging it would starve the pump for many
                # slots and turn every deadline into a lump.
                while tag not in _done_tags and filler_q:
                    c, t, fn = filler_q.pop(0)
                    fn()
                    _done_tags.add(t)

            # prelude: the critical chain to the first exp. kc0's t3-5
            # matmuls wait the second xTk half, so the q01(0) matmuls are
            # interleaved between (they only need xT quarter 0 + wq).
            ps_k0 = pspool.tile([P, 2, QB], f32, name="ps_k0", tag="ps")
            ps_q0 = pspool.tile([P, 2, QB], f32, name="ps_q0", tag="ps")
            o0, sz0 = kchunks[0]
            for t in range(3):
                nc.tensor.matmul(
                    ps_k0[:, 0, 0:sz0], wkq[:, t, 0:P], xTk[:, t, o0 : o0 + sz0],
                    start=(t == 0), stop=False,
                )
            for t in range(6):
                nc.tensor.matmul(
                    ps_q0[:, 0, :], wkq[:, 6 + t, 0:P], xT[:, 0, t, :],
                    start=(t == 0), stop=(t == 5),
                )
            for t in range(3, 6):
                nc.tensor.matmul(
                    ps_k0[:, 0, 0:sz0], wkq[:, t, 0:P], xTk[:, t, o0 : o0 + sz0],
                    start=False, stop=(t == 5),
                )
            nc.vector.tensor_copy(qT01[:, 0:QB], ps_q0[:, 0, :])
            nc.vector.tensor_copy(kT01[:, o0 : o0 + sz0], ps_k0[:, 0, 0:sz0])

            def pump(extra=0.0):
                budget[0] += extra
                while filler_q and budget[0] >= filler_q[0][0] - 1400:
                    c, t, fn = filler_q.pop(0)
                    fn()
                    _done_tags.add(t)
                    budget[0] -= c

            for s in range(len(SL) + 2):
                if s >= 2:
                    emit_av(SL[s - 2])
                if s < len(SL):
                    emit_sc(SL[s])
                pump(500.0)

            # tail: flush remaining fillers (the last projections)
            while filler_q:
                _, _t, fn = filler_q.pop(0)
                fn()

    nc.finalize()
    return nc


def _prep_inputs(x, mask, qkv_w, proj_w):
    """Build the 8 per-core input maps. Returns (in_maps, KP)."""
    idx = [np.nonzero(mask[b] == 0.0)[0] for b in range(B)]
    nk = max(len(i) for i in idx)
    KP = max(P, int(math.ceil(nk / P)) * P)
    JG = KP // P

    per_batch = []
    for b in range(B):
        xTb = np.ascontiguousarray(x[b].T)  # [C, N] f32
        xT_in = np.ascontiguousarray(
            xTb.reshape(6, P, NQB, QB).transpose(1, 2, 0, 3)
        ).astype(BF16)  # [P, NQB, 6, QB] -- contiguous 6KB line per quarter
        xk = np.zeros((C, KP), np.float32)
        xk[:, : len(idx[b])] = xTb[:, idx[b]]
        xTk_in = xk.reshape(6, P, KP).transpose(1, 0, 2).astype(BF16)
        kfv = np.zeros((KP,), np.float32)
        kfv[: len(idx[b])] = 1.0
        kfL_in = np.ascontiguousarray(
            np.broadcast_to(
                kfv[(JG - 1) * P :][:, None, None], (P, HPC, D)
            )
        ).astype(BF16)
        per_batch.append((xT_in, xTk_in, kfL_in))

    in_maps = []
    for c in range(NCORES):
        b, g = c // 4, c % 4
        h0 = HPC * g
        xT_in, xTk_in, kfL_in = per_batch[b]
        m = {"xT": xT_in, "xTk": xTk_in, "kfL": kfL_in}
        ws = {}
        for name, off in (("wqT", 0), ("wkT", C), ("wvT", 2 * C)):
            w = qkv_w[off + h0 * D : off + (h0 + HPC) * D]  # [192, C]
            ws[name] = (
                np.ascontiguousarray(w.T).reshape(6, P, 192).transpose(1, 0, 2).astype(BF16)
            )
        m["wkqT"] = np.ascontiguousarray(
            np.concatenate([ws["wkT"], ws["wqT"]], axis=1)
        )
        m["wvT"] = ws["wvT"]
        pw = proj_w[:, h0 * D : h0 * D + HPC * D]  # [768, 192]
        m["pT01"] = np.ascontiguousarray(pw[:, :P].T).reshape(P, 6, P).astype(BF16)
        pT2o = np.ascontiguousarray(pw[:, P:].T).reshape(D, 6, P)  # [64, 6, 128]
        pT2pk = np.empty((P, 3, P), np.float32)
        for j in range(3):
            pT2pk[0:D, j] = pT2o[:, 2 * j]
            pT2pk[D:P, j] = pT2o[:, 2 * j + 1]
        m["pT2pk"] = pT2pk.astype(BF16)
        in_maps.append(m)
    return in_maps, KP


_CACHE = {}


def _get_program(KP):
    if KP not in _CACHE:
        _CACHE[KP] = _build_program(KP)
    return _CACHE[KP]


def _gather_output(results, proj_b):
    out = np.empty((B, N, C), np.float32)
    for b in range(B):
        acc = None
        for c in range(4 * b, 4 * b + 4):
            a = results[c]["outT"]  # [128, NQB, 6, QB] bf16
            a = np.asarray(a, np.float32).transpose(2, 0, 1, 3).reshape(C, N)
            acc = a if acc is None else acc + a
        out[b] = acc.T + proj_b[None, :]
    return out


def kernel(x, mask, qkv_w, proj_w, proj_b, _want_results=False):
    from concourse.bass_utils import run_bass_kernel_spmd

    x = np.asarray(x, np.float32)
    mask = np.asarray(mask, np.float32)
    qkv_w = np.asarray(qkv_w, np.float32)
    proj_w = np.asarray(proj_w, np.float32)
    proj_b = np.asarray(proj_b, np.float32)

    in_maps, KP = _prep_inputs(x, mask, qkv_w, proj_w)
    nc = _get_program(KP)
    res = run_bass_kernel_spmd(nc, in_maps, list(range(NCORES)))

    out = _gather_output(res.results, proj_b)
    if _want_results:
        return out, res
    return out


# revision 20
# speedup vs baseline: 1.0936x; 1.0239x over previous
"""Masked multi-head attention kernel for 8 Trainium2 NeuronCores.

Strategy (v2 — PE-packing rework of the v1 baseline):
  - 24 (batch, head) pairs sharded as: core c -> batch c//4, heads [3*(c%4) .. 3*(c%4)+2].
  - Key-padding mask handled by HOST-side gather: only unmasked key positions are
    shipped/computed. Padded key slots get zeroed K columns (scores=0 -> exp=1)
    and a 0 in the indicator slot of V, so they contribute nothing.
  - Softmax without max-subtraction (scores ~ N(0,1); masked keys excluded).
  - Row-sum of exp folded into the AV matmul via an indicator slot on V.
  - PE tile packing (the big v2 win): all D=64-contraction matmuls run as
    concurrent PAIRS in the two 64-row halves of the PE array
    (tile_position auto-derived from partition bases):
      * scores: head0 (partitions 0-63) paired with head1 (64-127) per key
        block; head2 paired with itself via duplicated kT2/qT2 (dup'd by
        SBUF->SBUF DMA, free on the DMA engines).
      * out-proj: the K=64 tail (head2 dims) packed as column-group pairs
        (even cg rows 0-63, odd cg rows 64-127) against duplicated OT2.
      * qT2/kT2 projections: M=64 outputs packed 2 query-chunks per pass
        via column tiling.
  - One exp per score pair: [128, 2*512] ACTIVATE (amortizes ACT overhead,
    covers both heads of the pair).
  - ACT table preloaded by a dummy exp at t=0; all DMAs issued on the
    Sync + GpSimd queues so the Scalar engine does nothing but exp.
  - V indicator slots built on-device (gpsimd memset) + a tiny DMA for the
    padded tail block.
  - PSUM: 4 banks score pairs (x2), 2 banks AV accumulators, 2 banks proj.
  - bf16 matmul inputs, fp32 PSUM accumulation, bf16 output partials
    (host sums the 4 partials per batch in fp32, adds proj_b).
"""

import math

import numpy as np
import ml_dtypes

BF16 = ml_dtypes.bfloat16
B, N, C = 2, 2048, 768
H = 12
D = 64
HPC = 3          # heads per core
P = 128
QB = 512         # query block
NQB = N // QB
SCALE = D ** -0.5
NCORES = 8


def _chunks(total, size=QB):
    return [(o, min(size, total - o)) for o in range(0, total, size)]


def _build_program(KP: int):
    from concourse import bacc, mybir
    from concourse.tile import TileContext

    JG = KP // P
    f32 = mybir.dt.float32
    bf16 = mybir.dt.bfloat16
    Exp = mybir.ActivationFunctionType.Exp
    nc = bacc.Bacc(None, target_bir_lowering=False)

    xT_d = nc.declare_dram_parameter("xT", [P, NQB, 6, QB], bf16, False)
    xTk_d = nc.declare_dram_parameter("xTk", [P, 6, KP], bf16, False)
    kfL_d = nc.declare_dram_parameter("kfL", [P, HPC, D], bf16, False)
    wkq_d = nc.declare_dram_parameter("wkqT", [P, 12, 192], bf16, False)
    wv_d = nc.declare_dram_parameter("wvT", [P, 6, 192], bf16, False)
    pT01_d = nc.declare_dram_parameter("pT01", [P, 6, P], bf16, False)
    pT2_d = nc.declare_dram_parameter("pT2pk", [P, 3, P], bf16, False)
    out_d = nc.declare_dram_parameter("outT", [P, NQB, 6, QB], bf16, True)

    NHALF = N // 2

    with TileContext(nc) as tc:
        with (
            tc.tile_pool(name="const", bufs=1) as cpool,
            tc.tile_pool(name="work", bufs=1) as wpool,
            tc.tile_pool(name="pt", bufs=14) as ptpool,
            tc.tile_pool(name="rb", bufs=3) as rbpool,
            tc.tile_pool(name="outp", bufs=2) as opool,
            tc.tile_pool(name="ps", bufs=2, space="PSUM") as pspool,
            tc.tile_pool(name="po", bufs=2, space="PSUM") as popool,
            tc.tile_pool(name="pp", bufs=1, space="PSUM") as pppool,
        ):
            # ---- PE warm-up: dummy matmuls with no data deps keep the
            # PE busy through the DMA phase so HAM un-throttles (K=8/8)
            # before the first real projection matmul.
            warm_i = cpool.tile([1, 8], f32)
            warm_o = cpool.tile([1, 8], bf16)
            nc.vector.memset(warm_i[:], 0.0)
            # short PE warm-up with NON-ZERO data (zero operands may not
            # register as PE activity for the HAM clock gate)
            warm_w = cpool.tile([P, P], bf16)
            nc.vector.memset(warm_w[:], 1.0)
            pwarm = pspool.tile([P, 2, QB], f32, name="pwarm", tag="ps")
            for _ in range(12):
                nc.tensor.matmul(
                    pwarm[:, 0, 0:P], warm_w[:], warm_w[:], start=True, stop=True
                )

            # ---- constant tiles + input DMAs.
            # Scalar queue: small weight tensors only (done before first exp).
            # Sync/GpSimd queues: bulk x / xk traffic. xTk first (gates k/v
            # proj), then xT in 512-query quarters so the qb0 q-projection
            # unblocks as early as possible.
            wkq = cpool.tile([P, 12, 192], bf16)
            wv = cpool.tile([P, 6, 192], bf16)
            xT = cpool.tile([P, NQB, 6, QB], bf16)
            xTk = cpool.tile([P, 6, KP], bf16)
            pT01 = cpool.tile([P, 6, P], bf16)
            pT2 = cpool.tile([P, 3, P], bf16)

            # critical gates first: wk/wq on the scalar HWDGE ring, the two
            # xTk halves split across the sync/gpsimd rings, then the xT
            # quarters (one 3D DMA each), then everything else.
            # Critical set first -- wkq + xTk + xT quarter 0 gate the first
            # exp; everything else is held back behind a guard op on the
            # gpsimd queue so the critical transfers get the full HBM
            # bandwidth (all queues share ~358 GB/s).
            nc.scalar.dma_start(wkq[:], wkq_d[:])
            nc.sync.dma_start(xTk[:, 0:3, :], xTk_d[:, 0:3, :])
            nc.gpsimd.dma_start(xT[:, 0, :, :], xT_d[:, 0, :, :])
            nc.sync.dma_start(xTk[:, 3:6, :], xTk_d[:, 3:6, :])
            nc.scalar.dma_start(wv[:], wv_d[:])
            # exp-table preload: after the scalar-ring DMA issues so the
            # ~2.7us ACT_TABLE_LOAD doesn't delay them
            nc.scalar.activation(warm_o[:], warm_i[:], Exp)

            # ---- work tiles
            qT01 = wpool.tile([P, N], bf16)
            qT2d = wpool.tile([P, N], bf16)   # head2 q, duplicated in halves
            kT01 = wpool.tile([P, KP], bf16)
            kT2d = wpool.tile([P, KP], bf16)  # head2 k, duplicated in halves
            v_sb = wpool.tile([P, JG, 2 * HPC, D], bf16)
            OT01 = wpool.tile([P, NQB, QB], bf16)
            OT2d = wpool.tile([P, NQB, QB], bf16)  # head2 O^T, duplicated

            # V indicator slots: 1.0 everywhere except the padded tail block,
            # which comes from a tiny DMA.
            if JG > 1:
                nc.gpsimd.memset(v_sb[:, 0 : JG - 1, 0 : 2 * HPC : 2, :], 1.0)
            nc.scalar.dma_start(v_sb[:, JG - 1, 0 : 2 * HPC : 2, :], kfL_d[:])

            kchunks = _chunks(KP)

            # ---- projection building blocks (each emits its own PSUM tile
            # allocs; emission order = scheduler priority).
            def k01_chunk(o, sz):
                ps = pspool.tile([P, 2, QB], f32, name="psq", tag="ps")
                for t in range(6):
                    nc.tensor.matmul(
                        ps[:, 0, 0:sz], wkq[:, t, 0:P], xTk[:, t, o : o + sz],
                        start=(t == 0), stop=(t == 5),
                    )
                nc.vector.tensor_copy(kT01[:, o : o + sz], ps[:, 0, 0:sz])

            def k2_pair(pair):
                ps = pspool.tile([P, 2, QB], f32, name="psq2", tag="ps")
                (o0, sz0) = pair[0]
                for t in range(6):
                    nc.tensor.matmul(
                        ps[0:D, 0, 0:sz0], wkq[:, t, P:192],
                        xTk[:, t, o0 : o0 + sz0],
                        start=(t == 0), stop=(t == 5),
                    )
                    if len(pair) == 2:
                        (o1, sz1) = pair[1]
                        nc.tensor.matmul(
                            ps[D:P, 1, 0:sz1], wkq[:, t, P:192],
                            xTk[:, t, o1 : o1 + sz1],
                            start=(t == 0), stop=(t == 5),
                        )
                nc.vector.tensor_copy(kT2d[0:D, o0 : o0 + sz0], ps[0:D, 0, 0:sz0])
                if len(pair) == 2:
                    nc.vector.tensor_copy(
                        kT2d[0:D, pair[1][0] : pair[1][0] + pair[1][1]],
                        ps[D:P, 1, 0 : pair[1][1]],
                    )

            def q01_chunk(ci):
                o = ci * QB
                ps = pspool.tile([P, 2, QB], f32, name="psq", tag="ps")
                for t in range(6):
                    nc.tensor.matmul(
                        ps[:, 0, :], wkq[:, 6 + t, 0:P], xT[:, ci, t, :],
                        start=(t == 0), stop=(t == 5),
                    )
                nc.vector.tensor_copy(qT01[:, o : o + QB], ps[:, 0, :])

            def q2_pair(c0, c1):
                # two query chunks packed in column halves, dup'd by DMA;
                # c1 may equal c0 (single chunk -> direct duplicate, no DMA)
                ps = pspool.tile([P, 2, QB], f32, name="psq2", tag="ps")
                o0, o1 = c0 * QB, c1 * QB
                for t in range(6):
                    nc.tensor.matmul(
                        ps[0:D, 0, :], wkq[:, 6 + t, P:192], xT[:, c0, t, :],
                        start=(t == 0), stop=(t == 5),
                    )
                    if c1 != c0:
                        nc.tensor.matmul(
                            ps[D:P, 1, :], wkq[:, 6 + t, P:192], xT[:, c1, t, :],
                            start=(t == 0), stop=(t == 5),
                        )
                if c1 == c0:
                    nc.vector.tensor_copy(qT2d[0:D, o0 : o0 + QB], ps[0:D, 0, :])
                    nc.vector.tensor_copy(qT2d[D:P, o0 : o0 + QB], ps[0:D, 0, :])
                else:
                    nc.vector.tensor_copy(qT2d[0:D, o0 : o0 + QB], ps[0:D, 0, :])
                    nc.vector.tensor_copy(qT2d[0:D, o1 : o1 + QB], ps[D:P, 1, :])
                    nc.vector.tensor_copy(
                        qT2d[D:P, o0 : o1 + QB], qT2d[0:D, o0 : o1 + QB]
                    )

            def v_pair(i):
                blocks = [j for j in (i, i + 1) if j < JG]
                ps = pspool.tile([P, 2, QB], f32, name="psv", tag="ps")
                for s, j in enumerate(blocks):
                    for t in range(6):
                        nc.tensor.matmul(
                            ps[:, s, 0:192],
                            xTk[:, t, j * P : (j + 1) * P],
                            wv[:, t, :],
                            start=(t == 0), stop=(t == 5),
                        )
                for s, j in enumerate(blocks):
                    nc.vector.tensor_copy(
                        v_sb[:, j, 1 : 2 * HPC : 2, :], ps[:, s, 0:192]
                    )

            # ---- attention primitives: one "slot" = one score-pair +
            # one exp; the engine queues are static FIFO, so the emission
            # order below IS the runtime schedule. Per slot we emit the AV
            # matmuls of slot s-2 (whose exp has completed by then), the
            # scores of slot s, and ~500ns of budgeted filler work.
            pts = {}
            po_t = {}
            budget = [0.0]

            KC_TAGS = ["kc%d" % c for c in range(1, len(kchunks))]

            def emit_sc(key):
                qb, kind, i = key
                if kind == "a":
                    if qb >= 1:
                        need("q01_%d" % qb)
                    if i >= 4:
                        for t_ in KC_TAGS[: (i * P) // QB]:
                            need(t_)
                else:
                    need("k2")
                    need("q2_0" if qb <= 1 else "q2_%d" % qb)
                qs = slice(qb * QB, (qb + 1) * QB)
                ps = pspool.tile([P, 2, QB], f32, name="psc", tag="ps")
                pt = ptpool.tile([P, 2, QB], bf16, name="pt", tag="pt")
                if kind == "a":
                    jg = i
                    nc.tensor.matmul(
                        ps[:, 0, :], kT01[0:D, jg * P : (jg + 1) * P],
                        qT01[0:D, qs], start=True, stop=True,
                    )
                    nc.tensor.matmul(
                        ps[:, 1, :], kT01[D:P, jg * P : (jg + 1) * P],
                        qT01[D:P, qs], start=True, stop=True,
                    )
                    nc.scalar.activation(pt[:], ps[:], Exp, scale=float(SCALE))
                else:
                    j0, j1 = 2 * i, 2 * i + 1
                    hasb = j1 < JG
                    nc.tensor.matmul(
                        ps[:, 0, :], kT2d[0:D, j0 * P : (j0 + 1) * P],
                        qT2d[0:D, qs], start=True, stop=True,
                    )
                    if hasb:
                        nc.tensor.matmul(
                            ps[:, 1, :], kT2d[D:P, j1 * P : (j1 + 1) * P],
                            qT2d[D:P, qs], start=True, stop=True,
                        )
                        nc.scalar.activation(pt[:], ps[:], Exp, scale=float(SCALE))
                    else:
                        nc.scalar.activation(
                            pt[:, 0, :], ps[:, 0, :], Exp, scale=float(SCALE)
                        )
                pts[key] = pt

            NT2 = (JG + 1) // 2

            def emit_av(key):
                qb, kind, i = key
                if kind == "a":
                    need("v%d" % i)
                else:
                    need("v%d" % min(2 * i + 1, JG - 1))
                pt = pts.pop(key)
                if kind == "a":
                    if i == 0:
                        po_t[(qb, 0)] = popool.tile([P, QB], f32, name="po", tag="po")
                        po_t[(qb, 1)] = popool.tile([P, QB], f32, name="po", tag="po")
                    po0, po1 = po_t[(qb, 0)], po_t[(qb, 1)]
                    nc.tensor.matmul(
                        po0[:], v_sb[:, i, 0:2, :], pt[:, 0, :],
                        start=(i == 0), stop=(i == JG - 1),
                    )
                    nc.tensor.matmul(
                        po1[:], v_sb[:, i, 2:4, :], pt[:, 1, :],
                        start=(i == 0), stop=(i == JG - 1),
                    )
                    if i == JG - 1:
                        rb0 = rbpool.tile([D, QB], f32, tag="rb")
                        nc.vector.reciprocal_approx_fast(rb0[:], po0[0:D, :])
                        nc.vector.tensor_mul(OT01[0:D, qb, :], po0[D:P, :], rb0[:])
                        rb1 = rbpool.tile([D, QB], f32, tag="rb")
                        nc.vector.reciprocal_approx_fast(rb1[:], po1[0:D, :])
                        nc.vector.tensor_mul(OT01[D:P, qb, :], po1[D:P, :], rb1[:])
                else:
                    if i == 0:
                        po_t[(qb, 2)] = popool.tile([P, QB], f32, name="po", tag="po")
                    po2 = po_t[(qb, 2)]
                    j0, j1 = 2 * i, 2 * i + 1
                    hasb = j1 < JG
                    nc.tensor.matmul(
                        po2[:], v_sb[:, j0, 4:6, :], pt[:, 0, :],
                        start=(i == 0), stop=(i == NT2 - 1 and not hasb),
                    )
                    if hasb:
                        nc.tensor.matmul(
                            po2[:], v_sb[:, j1, 4:6, :], pt[:, 1, :],
                            start=False, stop=(i == NT2 - 1),
                        )
                    if i == NT2 - 1:
                        rb2 = rbpool.tile([D, QB], f32, tag="rb")
                        nc.vector.reciprocal_approx_fast(rb2[:], po2[0:D, :])
                        nc.vector.tensor_mul(OT2d[0:D, qb, :], po2[D:P, :], rb2[:])
                        nc.vector.tensor_mul(OT2d[D:P, qb, :], po2[D:P, :], rb2[:])
                        # projection for this query block becomes filler work
                        for j in range(3):
                            filler_q.insert(
                                j,
                                (800, "p%d_%d" % (qb, j),
                                 _mk_proj_unit(qb, j, last=(qb == 3))),
                            )

            ob_t = {}

            def _mk_proj_unit(qb, j, last=False):
                def emit():
                    if j == 0:
                        ob_t[qb] = opool.tile([P, 6, QB], bf16, name="ob", tag="ob")
                    ob = ob_t[qb]
                    # the final block's projection runs after the last exp:
                    # the score pool (4 banks) is free then, so use it to
                    # pipeline the three units instead of serializing on pp
                    if last:
                        pp = pspool.tile([P, 2, QB], f32, name="pp", tag="ps")
                    else:
                        pp = pppool.tile([P, 2, QB], f32, name="pp", tag="pp")
                    nc.tensor.matmul(
                        pp[:, 0, :], pT01[:, 2 * j, :], OT01[:, qb, :],
                        start=True, stop=False,
                    )
                    nc.tensor.matmul(
                        pp[:, 1, :], pT01[:, 2 * j + 1, :], OT01[:, qb, :],
                        start=True, stop=False,
                    )
                    nc.tensor.matmul(
                        pp[:, 0, :], pT2[0:D, j, :], OT2d[0:D, qb, :],
                        start=False, stop=True,
                    )
                    nc.tensor.matmul(
                        pp[:, 1, :], pT2[D:P, j, :], OT2d[D:P, qb, :],
                        start=False, stop=True,
                    )
                    if last and j != 1:
                        nc.scalar.copy(ob[:, 2 * j : 2 * j + 2, :], pp[:])
                    else:
                        nc.vector.tensor_copy(ob[:, 2 * j : 2 * j + 2, :], pp[:])
                    outq = nc.sync if qb % 2 == 0 else nc.gpsimd
                    outq.dma_start(
                        out_d[:, qb, 2 * j : 2 * j + 2, :],
                        ob[:, 2 * j : 2 * j + 2, :],
                    )
                return emit

            def v_block(j):
                def emit():
                    ps = pspool.tile([P, 2, QB], f32, name="psv", tag="ps")
                    for t in range(6):
                        nc.tensor.matmul(
                            ps[:, 0, 0:192],
                            xTk[:, t, j * P : (j + 1) * P],
                            wv[:, t, :],
                            start=(t == 0), stop=(t == 5),
                        )
                    nc.vector.tensor_copy(
                        v_sb[:, j, 1 : 2 * HPC : 2, :], ps[:, 0, 0:192]
                    )
                return emit

            def k2_all():
                for i in range(0, len(kchunks), 2):
                    k2_pair(kchunks[i : i + 2])
                nc.vector.tensor_copy(kT2d[D:P, :], kT2d[0:D, :])

            # ---- the static schedule.
            # stream block order: h01 of qb0..2, then their h2 blocks, then
            # qb3 -- this pushes the kT2d/qT2d deadline ~20us out so the
            # k2/q2 filler projections fit in the early slots.
            SL = []
            for qb, kind in (
                (0, "a"), (1, "a"), (2, "a"), (3, "a"),
                (0, "b"), (1, "b"), (2, "b"), (3, "b"),
            ):
                n = JG if kind == "a" else NT2
                for i in range(n):
                    SL.append((qb, kind, i))

            filler_q = []
            filler_q.append((660, "v0", v_block(0)))
            filler_q.append((660, "v1", v_block(1)))
            for ci, (o, sz) in enumerate(kchunks[1:]):
                filler_q.append(
                    (1400, "kc%d" % (ci + 1), lambda o=o, sz=sz: k01_chunk(o, sz))
                )
            for j in (2, 3, 4, 5):
                filler_q.append((660, "v%d" % j, v_block(j)))
            filler_q.append((1400, "q01_1", lambda: q01_chunk(1)))
            for j in range(6, JG):
                filler_q.append((660, "v%d" % j, v_block(j)))
            filler_q.append((2400, "k2", k2_all))
            filler_q.append((2400, "q2_0", lambda: q2_pair(0, 1)))
            filler_q.append((1400, "q01_2", lambda: q01_chunk(2)))
            filler_q.append((2800, "q2_2", lambda: q2_pair(2, 2)))
            filler_q.append((1400, "q01_3", lambda: q01_chunk(3)))
            filler_q.append((2800, "q2_3", lambda: q2_pair(3, 3)))
            _done_tags = set()

            def need(tag):
                # force-emit queued fillers (in order) until `tag` has run;
                # guarantees producers are emitted before their readers.
                # Does NOT charge the budget: a forced filler is already
                # late, and charging it would starve the pump for many
                # slots and turn every deadline into a lump.
                while tag not in _done_tags and filler_q:
                    c, t, fn = filler_q.pop(0)
                    fn()
                    _done_tags.add(t)

            # prelude: the critical chain to the first exp. kc0's t3-5
            # matmuls wait the second xTk half, so the q01(0) matmuls are
            # interleaved between (they only need xT quarter 0 + wq).
            ps_k0 = pspool.tile([P, 2, QB], f32, name="ps_k0", tag="ps")
            ps_q0 = pspool.tile([P, 2, QB], f32, name="ps_q0", tag="ps")
            o0, sz0 = kchunks[0]
            for t in range(3):
                nc.tensor.matmul(
                    ps_k0[:, 0, 0:sz0], wkq[:, t, 0:P], xTk[:, t, o0 : o0 + sz0],
                    start=(t == 0), stop=False,
                )
            for t in range(6):
                nc.tensor.matmul(
                    ps_q0[:, 0, :], wkq[:, 6 + t, 0:P], xT[:, 0, t, :],
                    start=(t == 0), stop=(t == 5),
                )
            for t in range(3, 6):
                nc.tensor.matmul(
                    ps_k0[:, 0, 0:sz0], wkq[:, t, 0:P], xTk[:, t, o0 : o0 + sz0],
                    start=False, stop=(t == 5),
                )
            nc.vector.tensor_copy(qT01[:, 0:QB], ps_q0[:, 0, :])
            nc.vector.tensor_copy(kT01[:, o0 : o0 + sz0], ps_k0[:, 0, 0:sz0])

            # guard: the gpsimd queue stalls here until the prelude q-proj
            # copy lands, releasing the bulk transfers only after the
            # critical ones are done
            guard = cpool.tile([1, 8], bf16)
            nc.gpsimd.tensor_copy(guard[:], qT01[0:1, 0:8])
            nc.gpsimd.dma_start(xT[:, 1, :, :], xT_d[:, 1, :, :])
            nc.gpsimd.dma_start(xT[:, 2, :, :], xT_d[:, 2, :, :])
            nc.gpsimd.dma_start(xT[:, 3, :, :], xT_d[:, 3, :, :])
            nc.sync.dma_start(pT01[:], pT01_d[:])
            nc.gpsimd.dma_start(pT2[:], pT2_d[:])

            def pump(extra=0.0):
                budget[0] += extra
                while filler_q and budget[0] >= filler_q[0][0] - 1400:
                    c, t, fn = filler_q.pop(0)
                    fn()
                    _done_tags.add(t)
                    budget[0] -= c

            for s in range(len(SL) + 2):
                if s >= 2:
                    emit_av(SL[s - 2])
                if s < len(SL):
                    emit_sc(SL[s])
                pump(500.0)

            # tail: flush remaining fillers (the last projections)
            while filler_q:
                _, _t, fn = filler_q.pop(0)
                fn()

    nc.finalize()
    return nc


def _prep_inputs(x, mask, qkv_w, proj_w):
    """Build the 8 per-core input maps. Returns (in_maps, KP)."""
    idx = [np.nonzero(mask[b] == 0.0)[0] for b in range(B)]
    nk = max(len(i) for i in idx)
    KP = max(P, int(math.ceil(nk / P)) * P)
    JG = KP // P

    per_batch = []
    for b in range(B):
        xTb = np.ascontiguousarray(x[b].T)  # [C, N] f32
        xT_in = np.ascontiguousarray(
            xTb.reshape(6, P, NQB, QB).transpose(1, 2, 0, 3)
        ).astype(BF16)  # [P, NQB, 6, QB] -- contiguous 6KB line per quarter
        xk = np.zeros((C, KP), np.float32)
        xk[:, : len(idx[b])] = xTb[:, idx[b]]
        xTk_in = xk.reshape(6, P, KP).transpose(1, 0, 2).astype(BF16)
        kfv = np.zeros((KP,), np.float32)
        kfv[: len(idx[b])] = 1.0
        kfL_in = np.ascontiguousarray(
            np.broadcast_to(
                kfv[(JG - 1) * P :][:, None, None], (P, HPC, D)
            )
        ).astype(BF16)
        per_batch.append((xT_in, xTk_in, kfL_in))

    in_maps = []
    for c in range(NCORES):
        b, g = c // 4, c % 4
        h0 = HPC * g
        xT_in, xTk_in, kfL_in = per_batch[b]
        m = {"xT": xT_in, "xTk": xTk_in, "kfL": kfL_in}
        ws = {}
        for name, off in (("wqT", 0), ("wkT", C), ("wvT", 2 * C)):
            w = qkv_w[off + h0 * D : off + (h0 + HPC) * D]  # [192, C]
            ws[name] = (
                np.ascontiguousarray(w.T).reshape(6, P, 192).transpose(1, 0, 2).astype(BF16)
            )
        m["wkqT"] = np.ascontiguousarray(
            np.concatenate([ws["wkT"], ws["wqT"]], axis=1)
        )
        m["wvT"] = ws["wvT"]
        pw = proj_w[:, h0 * D : h0 * D + HPC * D]  # [768, 192]
        m["pT01"] = np.ascontiguousarray(pw[:, :P].T).reshape(P, 6, P).astype(BF16)
        pT2o = np.ascontiguousarray(pw[:, P:].T).reshape(D, 6, P)  # [64, 6, 128]
        pT2pk = np.empty((P, 3, P), np.float32)
        for j in range(3):
            pT2pk[0:D, j] = pT2o[:, 2 * j]
            pT2pk[D:P, j] = pT2o[:, 2 * j + 1]
        m["pT2pk"] = pT2pk.astype(BF16)
        in_maps.append(m)
    return in_maps, KP


_CACHE = {}


def _get_program(KP):
    if KP not in _CACHE:
        _CACHE[KP] = _build_program(KP)
    return _CACHE[KP]


def _gather_output(results, proj_b):
    out = np.empty((B, N, C), np.float32)
    for b in range(B):
        acc = None
        for c in range(4 * b, 4 * b + 4):
            a = results[c]["outT"]  # [128, NQB, 6, QB] bf16
            a = np.asarray(a, np.float32).transpose(2, 0, 1, 3).reshape(C, N)
            acc = a if acc is None else acc + a
        out[b] = acc.T + proj_b[None, :]
    return out


def kernel(x, mask, qkv_w, proj_w, proj_b, _want_results=False):
    from concourse.bass_utils import run_bass_kernel_spmd

    x = np.asarray(x, np.float32)
    mask = np.asarray(mask, np.float32)
    qkv_w = np.asarray(qkv_w, np.float32)
    proj_w = np.asarray(proj_w, np.float32)
    proj_b = np.asarray(proj_b, np.float32)

    in_maps, KP = _prep_inputs(x, mask, qkv_w, proj_w)
    nc = _get_program(KP)
    res = run_bass_kernel_spmd(nc, in_maps, list(range(NCORES)))

    out = _gather_output(res.results, proj_b)
    if _want_results:
        return out, res
    return out
